# revision 21
# baseline (speedup 1.0000x reference)
"""CharLSTM (2-layer, H=256, B=512, T=512) Trainium2 Bass kernel.

Strategy: data-parallel over batch across 8 cores (64 batch/core).
Per core, a software-pipelined wavefront runs layer0 step t and layer1
step t-1 concurrently. Matmuls keep weights stationary (fp16), layout
[4H-on-partitions x batch-on-free], PSUM fp32.

Cell math (all-tanh form; minimizes ACT instructions and uses fused
DVE scalar_tensor_tensor ops). With sigma(x) = (tanh(x/2)+1)/2 and the
substitutions H := 2h, C := 2c, scale-folded weights give PSUM
pre-activations p = rowscale * a (rowscale = 0.5 for i,f,o rows; 1 for
g rows), so one tanh yields it = 2i-1, ft = 2f-1, ot = 2o-1, gt = g:

  T = (it + 1) * gt            # = 2 i*g
  U = (ft + 1) * C'            # = 4 f*c'
  C = U*0.5 + T                # = 2c
  tc = tanh(C * 0.5)           # = tanh(c)   (free scale on ACT)
  H = (ot + 1) * tc            # = 2h

All weights consuming h are pre-scaled by 0.5 on host. Layer-1 bias is
injected by a K=8 matmul (B1pack[8,128] @ I8-expand[8,512]) so no DVE
bias add is needed. Embedding lookup E[x] on host; layer-0 input
projection + bias is a K=9 matmul chunk [Wih0.T; b0] vs [emb_t; 1].
"""

import sys

sys.path.insert(0, "/opt/trn_rl_repo")

from contextlib import ExitStack

import numpy as np

VOCAB = 78
EMBED = 8
H = 256
BATCH = 512
SEQ = 512
NCORES = 8
BPC = BATCH // NCORES  # 64 batch per core
EBLK = 64  # emb prefetch block (steps)

_cache = {}


def _build_program(T):
    import concourse.tile as tile
    import concourse.mybir as mybir
    from concourse import bacc

    dt = mybir.dt
    AF = mybir.ActivationFunctionType
    ALU = mybir.AluOpType
    f32, f16 = dt.float32, dt.float16

    nc = bacc.Bacc("TRN2", target_bir_lowering=False, debug=False,
                   num_devices=NCORES)

    W0h_d = nc.dram_tensor("W0h", [128, 2, 1024], f16, kind="ExternalInput").ap()
    W0e_d = nc.dram_tensor("W0e", [9, 1024], f16, kind="ExternalInput").ap()
    W1_d = nc.dram_tensor("W1", [128, 4, 1024], f16, kind="ExternalInput").ap()
    B1p_d = nc.dram_tensor("B1p", [8, 128], f16, kind="ExternalInput").ap()
    E8_d = nc.dram_tensor("E8", [8, 512], f16, kind="ExternalInput").ap()
    Wfc_d = nc.dram_tensor("WfcT", [128, 2, VOCAB], f16, kind="ExternalInput").ap()
    bfc_d = nc.dram_tensor("bfc", [VOCAB, 1], f32, kind="ExternalInput").ap()
    emb_d = nc.dram_tensor("embT", [9, T * BPC], f16, kind="ExternalInput").ap()
    out_d = nc.dram_tensor("out", [VOCAB, BPC], f32, kind="ExternalOutput").ap()

    with tile.TileContext(nc) as tc, ExitStack() as ctx:
        const = ctx.enter_context(tc.tile_pool(name="const", bufs=1))
        W0h = const.tile([128, 2, 1024], f16)
        nc.sync.dma_start(W0h[:], W0h_d)
        W0e = const.tile([9, 1024], f16)
        nc.sync.dma_start(W0e[:], W0e_d)
        W1 = const.tile([128, 4, 1024], f16)
        nc.sync.dma_start(W1[:], W1_d)
        B1p = const.tile([8, 128], f16)
        nc.sync.dma_start(B1p[:], B1p_d)
        E8 = const.tile([8, 512], f16)
        nc.sync.dma_start(E8[:], E8_d)
        WfcT = const.tile([128, 2, VOCAB], f16)
        nc.sync.dma_start(WfcT[:], Wfc_d)
        bfc = const.tile([VOCAB, 1], f32)
        nc.sync.dma_start(bfc[:], bfc_d)

        embp = ctx.enter_context(tc.tile_pool(name="embp", bufs=2))
        ps0p = ctx.enter_context(tc.tile_pool(name="ps0p", bufs=3, space="PSUM"))
        ps1p = ctx.enter_context(tc.tile_pool(name="ps1p", bufs=3, space="PSUM"))
        psfcp = ctx.enter_context(tc.tile_pool(name="psfcp", bufs=1, space="PSUM"))
        gp = ctx.enter_context(tc.tile_pool(name="gp", bufs=2))
        op_ = ctx.enter_context(tc.tile_pool(name="op", bufs=2))
        up = ctx.enter_context(tc.tile_pool(name="up", bufs=2))
        tp = ctx.enter_context(tc.tile_pool(name="tp", bufs=2))
        cp = ctx.enter_context(tc.tile_pool(name="cp", bufs=2))
        tcp = ctx.enter_context(tc.tile_pool(name="tcp", bufs=2))
        hp = ctx.enter_context(tc.tile_pool(name="hp", bufs=2))
        fcp = ctx.enter_context(tc.tile_pool(name="fcp", bufs=1))

        eblk = min(EBLK, T)
        nblk = (T + eblk - 1) // eblk
        emb_tiles = [None] * nblk
        ps0_tiles = {}
        ps1_tiles = {}
        h0_prev = h1_prev = None

        def emit_bias(step):
            # ps1 bank for L1 step `step`-1 (wavefront `step`): allocate one
            # wavefront step early and inject b1 via a K=8 matmul so the PE
            # does it warm, inside the previous step's burst.
            ps1 = ps1p.tile([128, 512], f32, name="ps1")
            ps1_tiles[step] = ps1
            nc.tensor.matmul(ps1[:, 0:512], B1p[:, :], E8[:, :],
                             start=True, stop=False, skip_group_check=True)

        def emit_emb(step):
            # layer-0 input-projection matmuls for `step`, into a fresh ps0
            # tile, emitted 2 steps early as PE filler. m0 carries start=True
            # (clears bank has_written); later mms overwrite/accumulate per
            # element.
            ps0 = ps0p.tile([128, 512], f32, name="ps0")
            ps0_tiles[step] = ps0
            emb_sb = emb_tiles[step // eblk]
            erhs = emb_sb[:, (step % eblk) * BPC:(step % eblk + 1) * BPC]
            for m in range(8):
                nc.tensor.matmul(ps0[:, m * 64:(m + 1) * 64],
                                 W0e[:, m * 128:(m + 1) * 128], erhs,
                                 start=(m == 0),
                                 stop=(step == 0 and m == 7),
                                 skip_group_check=True)

        def cell(ps, first, tag):
            # gates (f,i,g,o) after the host row-permutation: tanh(f) first
            # (it alone gates U, the first chain op), then tanh(i,g), then
            # tanh(o) (only needed at the very end of the chain)
            Gf = gp.tile([128, 128], f16, name="Gf" + tag)
            nc.scalar.activation(Gf[:], ps[:, 0:128], AF.Tanh)
            Gig = gp.tile([128, 256], f16, name="Gig" + tag)
            nc.scalar.activation(Gig[:], ps[:, 128:384], AF.Tanh)
            O = op_.tile([128, 128], f16, name="O" + tag)
            nc.scalar.activation(O[:], ps[:, 384:512], AF.Tanh)
            C = cp.tile([128, 128], f16, name="C" + tag)
            if first:
                # c = i*g  ->  C = (it+1)*gt
                nc.vector.scalar_tensor_tensor(
                    C[:], Gig[:, 0:128], 1.0, Gig[:, 128:256],
                    ALU.add, ALU.mult)
            else:
                U = up.tile([128, 128], f16, name="U" + tag)
                nc.vector.scalar_tensor_tensor(
                    U[:], Gf[:], 1.0, cell.cprev[tag][:], ALU.add, ALU.mult)
                Tt = tp.tile([128, 128], f16, name="T" + tag)
                nc.vector.scalar_tensor_tensor(
                    Tt[:], Gig[:, 0:128], 1.0, Gig[:, 128:256],
                    ALU.add, ALU.mult)
                nc.vector.scalar_tensor_tensor(
                    C[:], U[:], 0.5, Tt[:], ALU.mult, ALU.add)
            TC = tcp.tile([128, 128], f16, name="TC" + tag)
            nc.scalar.activation(TC[:], C[:], AF.Tanh, scale=0.5)
            Hh = hp.tile([128, 128], f16, name="H" + tag)
            nc.vector.scalar_tensor_tensor(
                Hh[:], O[:], 1.0, TC[:], ALU.add, ALU.mult)
            cell.cprev[tag] = C
            return Hh

        cell.cprev = {}

        for s in range(T + 1):
            # prefetch emb blocks: block 0 at s=0, block b+1 at start of block b
            if s < T and s % eblk == 0:
                b = s // eblk
                if b == 0:
                    e0 = embp.tile([9, eblk * BPC], f16, name="embblk")
                    nc.sync.dma_start(e0[:], emb_d[:, 0:eblk * BPC])
                    emb_tiles[0] = e0
                if b + 1 < nblk:
                    e1 = embp.tile([9, eblk * BPC], f16, name="embblk")
                    nc.sync.dma_start(
                        e1[:],
                        emb_d[:, (b + 1) * eblk * BPC:(b + 2) * eblk * BPC])
                    emb_tiles[b + 1] = e1

            h0_in = h0_prev  # H0(s-1), consumed by both layers this step
            h1_in = h1_prev  # H1(s-2)

            if s == 0:
                emit_emb(0)
                if T > 1:
                    emit_emb(1)
                emit_bias(1)
            # emb/bias filler emitted at the TOP of the step: they are
            # dependency-free so the in-order PE queue runs them during the
            # H0(s-1) wait, keeping the PE warm across the step boundary.
            if 1 <= s and s + 1 <= T:
                emit_bias(s + 1)
            if s + 2 < T:
                emit_emb(s + 2)

            if s < T:
                # ---- layer 0, step s ----
                ps0 = ps0_tiles.pop(s)
                with tc.high_priority():
                    if s > 0:
                        for m in range(8):
                            o = ps0[:, m * 64:(m + 1) * 64]
                            for k in range(2):
                                nc.tensor.matmul(
                                    o, W0h[:, k, m * 128:(m + 1) * 128],
                                    h0_in[:, k * 64:(k + 1) * 64],
                                    start=False, stop=(m == 7 and k == 1),
                                    skip_group_check=True)
                    h0_prev = cell(ps0, s == 0, "0")

            if s > 0:
                # ---- layer 1, step s-1 (bias already in ps1, injected one
                # wavefront step early) ----
                ps1 = ps1_tiles.pop(s)
                for m in range(8):
                    o = ps1[:, m * 64:(m + 1) * 64]
                    if s > 1:
                        for k in range(2):
                            nc.tensor.matmul(
                                o, W1[:, k, m * 128:(m + 1) * 128],
                                h1_in[:, k * 64:(k + 1) * 64],
                                start=False, stop=False,
                                skip_group_check=True)
                    for k in range(2):
                        nc.tensor.matmul(
                            o, W1[:, 2 + k, m * 128:(m + 1) * 128],
                            h0_in[:, k * 64:(k + 1) * 64],
                            start=False, stop=(m == 7 and k == 1),
                            skip_group_check=True)
                h1_prev = cell(ps1, s == 1, "1")

        # ---- final FC on h1(T-1) ----
        psfc = psfcp.tile([VOCAB, BPC], f32)
        for k in range(2):
            nc.tensor.matmul(psfc[:], WfcT[:, k, :],
                             h1_prev[:, k * 64:(k + 1) * 64],
                             start=(k == 0), stop=(k == 1))
        fc = fcp.tile([VOCAB, BPC], f32)
        nc.scalar.activation(fc[:], psfc[:], AF.Identity, bias=bfc[:])
        nc.sync.dma_start(out_d, fc[:])

    nc.compile()
    return nc


def _prep_inputs(x, E, Wih0, Whh0, bih0, bhh0, Wih1, Whh1, bih1, bhh1,
                 Wfc, bfc, T):
    """Host-side weight folding and per-core input shards."""
    f16 = np.float16
    # permute gate rows (i,f,g,o) -> (f,i,g,o) so tanh(f) is a single leading
    # block (it alone gates the first chain op U)
    perm = np.r_[256:512, 0:256, 512:1024]
    # rowscale: 0.5 for f,i,o gate rows (sigma -> tanh), 1.0 for g rows
    rs = np.ones(1024, np.float32)
    rs[0:512] = 0.5      # f, i
    rs[768:1024] = 0.5   # o
    Wih0 = np.asarray(Wih0, np.float32)[perm] * rs[:, None]
    Whh0 = np.asarray(Whh0, np.float32)[perm] * rs[:, None] * 0.5  # H0 = 2h0
    b0 = (np.asarray(bih0, np.float32) + np.asarray(bhh0, np.float32))[perm] * rs
    Wih1 = np.asarray(Wih1, np.float32)[perm] * rs[:, None] * 0.5  # H0 = 2h0
    Whh1 = np.asarray(Whh1, np.float32)[perm] * rs[:, None] * 0.5  # H1 = 2h1
    b1 = (np.asarray(bih1, np.float32) + np.asarray(bhh1, np.float32))[perm] * rs
    Wfc = np.asarray(Wfc, np.float32) * 0.5                  # input H1 = 2h1
    bfc = np.asarray(bfc, np.float32)

    W0h = np.ascontiguousarray(
        Whh0.T.reshape(2, 128, 1024).transpose(1, 0, 2)).astype(f16)
    W0e = np.concatenate([Wih0.T, b0[None, :]], axis=0).astype(f16)  # [9,1024]
    W1 = np.ascontiguousarray(
        np.concatenate([Whh1.T, Wih1.T], axis=0)  # [512, 1024]
        .reshape(4, 128, 1024).transpose(1, 0, 2)).astype(f16)
    B1p = np.ascontiguousarray(b1.reshape(8, 128)).astype(f16)
    E8 = np.ascontiguousarray(
        np.repeat(np.eye(8, dtype=np.float32), 64, axis=1)).astype(f16)
    WfcT = np.ascontiguousarray(
        Wfc.T.reshape(2, 128, VOCAB).transpose(1, 0, 2)).astype(f16)
    bfc2 = np.ascontiguousarray(bfc[:, None]).astype(np.float32)

    E2 = np.asarray(E, np.float32).copy()
    E2[0] = 0.0  # padding_idx=0
    x = np.asarray(x)

    common = {"W0h": W0h, "W0e": W0e, "W1": W1, "B1p": B1p, "E8": E8,
              "WfcT": WfcT, "bfc": bfc2}
    in_maps = []
    for i in range(NCORES):
        xs = x[i * BPC:(i + 1) * BPC, :T]  # [64, T]
        emb = E2[xs]  # [64, T, 8]
        embT = np.empty((9, T, BPC), np.float32)
        embT[:8] = emb.transpose(2, 1, 0)
        embT[8] = 1.0
        m = dict(common)
        m["embT"] = np.ascontiguousarray(embT.reshape(9, T * BPC)).astype(f16)
        in_maps.append(m)
    return in_maps


def kernel(x, E, Wih0, Whh0, bih0, bhh0, Wih1, Whh1, bih1, bhh1, Wfc, bfc,
           T=SEQ, trace=False):
    from concourse import bass_utils

    if T not in _cache:
        _cache[T] = _build_program(T)
    nc = _cache[T]
    in_maps = _prep_inputs(x, E, Wih0, Whh0, bih0, bhh0, Wih1, Whh1, bih1,
                           bhh1, Wfc, bfc, T)
    res = bass_utils.run_bass_kernel_spmd(nc, in_maps, list(range(NCORES)),
                                          trace=trace)
    out = np.empty((BATCH, VOCAB), np.float32)
    for i in range(NCORES):
        out[i * BPC:(i + 1) * BPC] = np.asarray(res.results[i]["out"]).T
    if trace:
        return out, res
    return out


# revision 22
# speedup vs baseline: 1.0295x; 1.0295x over previous
"""CharLSTM (2-layer, H=256, B=512, T=512) Trainium2 Bass kernel.

Strategy: data-parallel over batch across 8 cores (64 batch/core).
Per core, a software-pipelined wavefront runs layer0 step t and layer1
step t-1 concurrently. Matmuls keep weights stationary (fp16), layout
[4H-on-partitions x batch-on-free], PSUM fp32.

Cell math (all-tanh form; minimizes ACT instructions and uses fused
DVE scalar_tensor_tensor ops). With sigma(x) = (tanh(x/2)+1)/2 and the
substitutions H := 2h, C := 2c, scale-folded weights give PSUM
pre-activations p = rowscale * a (rowscale = 0.5 for i,f,o rows; 1 for
g rows), so one tanh yields it = 2i-1, ft = 2f-1, ot = 2o-1, gt = g:

  T = (it + 1) * gt            # = 2 i*g
  U = (ft + 1) * C'            # = 4 f*c'
  C = U*0.5 + T                # = 2c
  tc = tanh(C * 0.5)           # = tanh(c)   (free scale on ACT)
  H = (ot + 1) * tc            # = 2h

All weights consuming h are pre-scaled by 0.5 on host. Layer-1 bias is
injected by a K=8 matmul (B1pack[8,128] @ I8-expand[8,512]) so no DVE
bias add is needed. Embedding lookup E[x] on host; layer-0 input
projection + bias is a K=9 matmul chunk [Wih0.T; b0] vs [emb_t; 1].
"""

import sys

sys.path.insert(0, "/opt/trn_rl_repo")

from contextlib import ExitStack

import numpy as np

VOCAB = 78
EMBED = 8
H = 256
BATCH = 512
SEQ = 512
NCORES = 8
BPC = BATCH // NCORES  # 64 batch per core
EBLK = 64  # emb prefetch block (steps)

_cache = {}


def _build_program(T):
    import concourse.tile as tile
    import concourse.mybir as mybir
    from concourse import bacc

    dt = mybir.dt
    AF = mybir.ActivationFunctionType
    ALU = mybir.AluOpType
    f32, f16 = dt.float32, dt.float16

    nc = bacc.Bacc("TRN2", target_bir_lowering=False, debug=False,
                   num_devices=NCORES)

    W0h_d = nc.dram_tensor("W0h", [128, 2, 1024], f16, kind="ExternalInput").ap()
    W0e_d = nc.dram_tensor("W0e", [9, 1024], f16, kind="ExternalInput").ap()
    W1_d = nc.dram_tensor("W1", [128, 4, 1024], f16, kind="ExternalInput").ap()
    B1p_d = nc.dram_tensor("B1p", [8, 128], f16, kind="ExternalInput").ap()
    E8_d = nc.dram_tensor("E8", [8, 512], f16, kind="ExternalInput").ap()
    Wfc_d = nc.dram_tensor("WfcT", [128, 2, VOCAB], f16, kind="ExternalInput").ap()
    bfc_d = nc.dram_tensor("bfc", [VOCAB, 1], f32, kind="ExternalInput").ap()
    emb_d = nc.dram_tensor("embT", [9, T * BPC], f16, kind="ExternalInput").ap()
    out_d = nc.dram_tensor("out", [VOCAB, BPC], f32, kind="ExternalOutput").ap()

    with tile.TileContext(nc) as tc, ExitStack() as ctx:
        const = ctx.enter_context(tc.tile_pool(name="const", bufs=1))
        W0h = const.tile([128, 2, 1024], f16)
        nc.sync.dma_start(W0h[:], W0h_d)
        W0e = const.tile([9, 1024], f16)
        nc.sync.dma_start(W0e[:], W0e_d)
        W1 = const.tile([128, 4, 1024], f16)
        nc.sync.dma_start(W1[:], W1_d)
        B1p = const.tile([8, 128], f16)
        nc.sync.dma_start(B1p[:], B1p_d)
        E8 = const.tile([8, 512], f16)
        nc.sync.dma_start(E8[:], E8_d)
        WfcT = const.tile([128, 2, VOCAB], f16)
        nc.sync.dma_start(WfcT[:], Wfc_d)
        bfc = const.tile([VOCAB, 1], f32)
        nc.sync.dma_start(bfc[:], bfc_d)

        embp = ctx.enter_context(tc.tile_pool(name="embp", bufs=2))
        ps0p = ctx.enter_context(tc.tile_pool(name="ps0p", bufs=3, space="PSUM"))
        ps1p = ctx.enter_context(tc.tile_pool(name="ps1p", bufs=2, space="PSUM"))
        psfcp = ctx.enter_context(tc.tile_pool(name="psfcp", bufs=1, space="PSUM"))
        gp = ctx.enter_context(tc.tile_pool(name="gp", bufs=2))
        op_ = ctx.enter_context(tc.tile_pool(name="op", bufs=2))
        up = ctx.enter_context(tc.tile_pool(name="up", bufs=2))
        tp = ctx.enter_context(tc.tile_pool(name="tp", bufs=2))
        cp = ctx.enter_context(tc.tile_pool(name="cp", bufs=2))
        tcp = ctx.enter_context(tc.tile_pool(name="tcp", bufs=2))
        hp = ctx.enter_context(tc.tile_pool(name="hp", bufs=2))
        fcp = ctx.enter_context(tc.tile_pool(name="fcp", bufs=1))

        eblk = min(EBLK, T)
        nblk = (T + eblk - 1) // eblk
        emb_tiles = [None] * nblk
        ps0_tiles = {}
        ps1_tiles = {}
        h0_prev = h1_prev = None

        def emit_bias(step):
            # ps1 bank for L1 step `step`-1 (wavefront `step`): allocate one
            # wavefront step early and inject b1 via a K=8 matmul so the PE
            # does it warm, inside the previous step's burst.
            ps1 = ps1p.tile([128, 512], f32, name="ps1")
            ps1_tiles[step] = ps1
            nc.tensor.matmul(ps1[:, 0:512], B1p[:, :], E8[:, :],
                             start=True, stop=False, skip_group_check=True)

        def emit_emb(step):
            # layer-0 input-projection matmuls for `step`, into a fresh ps0
            # tile, emitted 2 steps early as PE filler. m0 carries start=True
            # (clears bank has_written); later mms overwrite/accumulate per
            # element.
            ps0 = ps0p.tile([128, 512], f32, name="ps0")
            ps0_tiles[step] = ps0
            emb_sb = emb_tiles[step // eblk]
            erhs = emb_sb[:, (step % eblk) * BPC:(step % eblk + 1) * BPC]
            for m in range(8):
                nc.tensor.matmul(ps0[:, m * 64:(m + 1) * 64],
                                 W0e[:, m * 128:(m + 1) * 128], erhs,
                                 start=(m == 0),
                                 stop=(step == 0 and m == 7),
                                 skip_group_check=True)

        def cell(ps, first, tag):
            # gates (f,i,g,o) after the host row-permutation: tanh(f) first
            # (it alone gates U, the first chain op), then tanh(i,g), then
            # tanh(o) (only needed at the very end of the chain)
            Gf = gp.tile([128, 128], f16, name="Gf" + tag)
            nc.scalar.activation(Gf[:], ps[:, 0:128], AF.Tanh)
            Gig = gp.tile([128, 256], f16, name="Gig" + tag)
            nc.scalar.activation(Gig[:], ps[:, 128:384], AF.Tanh)
            O = op_.tile([128, 128], f16, name="O" + tag)
            nc.scalar.activation(O[:], ps[:, 384:512], AF.Tanh)
            C = cp.tile([128, 128], f16, name="C" + tag)
            if first:
                # c = i*g  ->  C = (it+1)*gt
                nc.vector.scalar_tensor_tensor(
                    C[:], Gig[:, 0:128], 1.0, Gig[:, 128:256],
                    ALU.add, ALU.mult)
            else:
                U = up.tile([128, 128], f16, name="U" + tag)
                nc.vector.scalar_tensor_tensor(
                    U[:], Gf[:], 1.0, cell.cprev[tag][:], ALU.add, ALU.mult)
                Tt = tp.tile([128, 128], f16, name="T" + tag)
                nc.vector.scalar_tensor_tensor(
                    Tt[:], Gig[:, 0:128], 1.0, Gig[:, 128:256],
                    ALU.add, ALU.mult)
                nc.vector.scalar_tensor_tensor(
                    C[:], U[:], 0.5, Tt[:], ALU.mult, ALU.add)
            TC = tcp.tile([128, 128], f16, name="TC" + tag)
            nc.scalar.activation(TC[:], C[:], AF.Tanh, scale=0.5)
            Hh = hp.tile([128, 128], f16, name="H" + tag)
            nc.vector.scalar_tensor_tensor(
                Hh[:], O[:], 1.0, TC[:], ALU.add, ALU.mult)
            cell.cprev[tag] = C
            return Hh

        cell.cprev = {}

        for s in range(T + 1):
            # prefetch emb blocks: block 0 at s=0, block b+1 at start of block b
            if s < T and s % eblk == 0:
                b = s // eblk
                if b == 0:
                    e0 = embp.tile([9, eblk * BPC], f16, name="embblk")
                    nc.sync.dma_start(e0[:], emb_d[:, 0:eblk * BPC])
                    emb_tiles[0] = e0
                if b + 1 < nblk:
                    e1 = embp.tile([9, eblk * BPC], f16, name="embblk")
                    nc.sync.dma_start(
                        e1[:],
                        emb_d[:, (b + 1) * eblk * BPC:(b + 2) * eblk * BPC])
                    emb_tiles[b + 1] = e1

            h0_in = h0_prev  # H0(s-1), consumed by both layers this step
            h1_in = h1_prev  # H1(s-2)

            if s == 0:
                emit_emb(0)
                if T > 1:
                    emit_emb(1)
                emit_bias(1)
            # emb/bias filler emitted at the TOP of the step: they are
            # dependency-free so the in-order PE queue runs them during the
            # H0(s-1) wait, keeping the PE warm across the step boundary.
            if 1 <= s and s + 1 <= T:
                emit_bias(s + 1)
            if s + 2 < T:
                emit_emb(s + 2)

            if s < T:
                # ---- layer 0, step s ----
                ps0 = ps0_tiles.pop(s)
                with tc.high_priority():
                    if s > 0:
                        for m in range(8):
                            o = ps0[:, m * 64:(m + 1) * 64]
                            for k in range(2):
                                nc.tensor.matmul(
                                    o, W0h[:, k, m * 128:(m + 1) * 128],
                                    h0_in[:, k * 64:(k + 1) * 64],
                                    start=False, stop=(m == 7 and k == 1),
                                    skip_group_check=True)
                    h0_prev = cell(ps0, s == 0, "0")

            if s > 0:
                # ---- layer 1, step s-1 (bias already in ps1, injected one
                # wavefront step early) ----
                ps1 = ps1_tiles.pop(s)
                for m in range(8):
                    o = ps1[:, m * 64:(m + 1) * 64]
                    if s > 1:
                        for k in range(2):
                            nc.tensor.matmul(
                                o, W1[:, k, m * 128:(m + 1) * 128],
                                h1_in[:, k * 64:(k + 1) * 64],
                                start=False, stop=False,
                                skip_group_check=True)
                    for k in range(2):
                        nc.tensor.matmul(
                            o, W1[:, 2 + k, m * 128:(m + 1) * 128],
                            h0_in[:, k * 64:(k + 1) * 64],
                            start=False, stop=(m == 7 and k == 1),
                            skip_group_check=True)
                h1_prev = cell(ps1, s == 1, "1")

        # ---- final FC on h1(T-1) ----
        psfc = psfcp.tile([VOCAB, BPC], f32)
        for k in range(2):
            nc.tensor.matmul(psfc[:], WfcT[:, k, :],
                             h1_prev[:, k * 64:(k + 1) * 64],
                             start=(k == 0), stop=(k == 1))
        fc = fcp.tile([VOCAB, BPC], f32)
        nc.scalar.activation(fc[:], psfc[:], AF.Identity, bias=bfc[:])
        nc.sync.dma_start(out_d, fc[:])

    nc.compile()
    return nc


def _prep_inputs(x, E, Wih0, Whh0, bih0, bhh0, Wih1, Whh1, bih1, bhh1,
                 Wfc, bfc, T):
    """Host-side weight folding and per-core input shards."""
    f16 = np.float16
    # permute gate rows (i,f,g,o) -> (f,i,g,o) so tanh(f) is a single leading
    # block (it alone gates the first chain op U)
    perm = np.r_[256:512, 0:256, 512:1024]
    # rowscale: 0.5 for f,i,o gate rows (sigma -> tanh), 1.0 for g rows
    rs = np.ones(1024, np.float32)
    rs[0:512] = 0.5      # f, i
    rs[768:1024] = 0.5   # o
    Wih0 = np.asarray(Wih0, np.float32)[perm] * rs[:, None]
    Whh0 = np.asarray(Whh0, np.float32)[perm] * rs[:, None] * 0.5  # H0 = 2h0
    b0 = (np.asarray(bih0, np.float32) + np.asarray(bhh0, np.float32))[perm] * rs
    Wih1 = np.asarray(Wih1, np.float32)[perm] * rs[:, None] * 0.5  # H0 = 2h0
    Whh1 = np.asarray(Whh1, np.float32)[perm] * rs[:, None] * 0.5  # H1 = 2h1
    b1 = (np.asarray(bih1, np.float32) + np.asarray(bhh1, np.float32))[perm] * rs
    Wfc = np.asarray(Wfc, np.float32) * 0.5                  # input H1 = 2h1
    bfc = np.asarray(bfc, np.float32)

    W0h = np.ascontiguousarray(
        Whh0.T.reshape(2, 128, 1024).transpose(1, 0, 2)).astype(f16)
    W0e = np.concatenate([Wih0.T, b0[None, :]], axis=0).astype(f16)  # [9,1024]
    W1 = np.ascontiguousarray(
        np.concatenate([Whh1.T, Wih1.T], axis=0)  # [512, 1024]
        .reshape(4, 128, 1024).transpose(1, 0, 2)).astype(f16)
    B1p = np.ascontiguousarray(b1.reshape(8, 128)).astype(f16)
    E8 = np.ascontiguousarray(
        np.repeat(np.eye(8, dtype=np.float32), 64, axis=1)).astype(f16)
    WfcT = np.ascontiguousarray(
        Wfc.T.reshape(2, 128, VOCAB).transpose(1, 0, 2)).astype(f16)
    bfc2 = np.ascontiguousarray(bfc[:, None]).astype(np.float32)

    E2 = np.asarray(E, np.float32).copy()
    E2[0] = 0.0  # padding_idx=0
    x = np.asarray(x)

    common = {"W0h": W0h, "W0e": W0e, "W1": W1, "B1p": B1p, "E8": E8,
              "WfcT": WfcT, "bfc": bfc2}
    in_maps = []
    for i in range(NCORES):
        xs = x[i * BPC:(i + 1) * BPC, :T]  # [64, T]
        emb = E2[xs]  # [64, T, 8]
        embT = np.empty((9, T, BPC), np.float32)
        embT[:8] = emb.transpose(2, 1, 0)
        embT[8] = 1.0
        m = dict(common)
        m["embT"] = np.ascontiguousarray(embT.reshape(9, T * BPC)).astype(f16)
        in_maps.append(m)
    return in_maps


def kernel(x, E, Wih0, Whh0, bih0, bhh0, Wih1, Whh1, bih1, bhh1, Wfc, bfc,
           T=SEQ, trace=False):
    from concourse import bass_utils

    if T not in _cache:
        _cache[T] = _build_program(T)
    nc = _cache[T]
    in_maps = _prep_inputs(x, E, Wih0, Whh0, bih0, bhh0, Wih1, Whh1, bih1,
                           bhh1, Wfc, bfc, T)
    res = bass_utils.run_bass_kernel_spmd(nc, in_maps, list(range(NCORES)),
                                          trace=trace)
    out = np.empty((BATCH, VOCAB), np.float32)
    for i in range(NCORES):
        out[i * BPC:(i + 1) * BPC] = np.asarray(res.results[i]["out"]).T
    if trace:
        return out, res
    return out


# revision 24
# speedup vs baseline: 1.0349x; 1.0053x over previous
"""CharLSTM (2-layer, H=256, B=512, T=512) Trainium2 Bass kernel.

Strategy: data-parallel over batch across 8 cores (64 batch/core).
Per core, a software-pipelined wavefront runs layer0 step t and layer1
step t-1 concurrently. Matmuls keep weights stationary (fp16), layout
[4H-on-partitions x batch-on-free], PSUM fp32.

Cell math (all-tanh form; minimizes ACT instructions and uses fused
DVE scalar_tensor_tensor ops). With sigma(x) = (tanh(x/2)+1)/2 and the
substitutions H := 2h, C := 2c, scale-folded weights give PSUM
pre-activations p = rowscale * a (rowscale = 0.5 for i,f,o rows; 1 for
g rows), so one tanh yields it = 2i-1, ft = 2f-1, ot = 2o-1, gt = g:

  T = (it + 1) * gt            # = 2 i*g
  U = (ft + 1) * C'            # = 4 f*c'
  C = U*0.5 + T                # = 2c
  tc = tanh(C * 0.5)           # = tanh(c)   (free scale on ACT)
  H = (ot + 1) * tc            # = 2h

All weights consuming h are pre-scaled by 0.5 on host. Layer-1 bias is
injected by a K=8 matmul (B1pack[8,128] @ I8-expand[8,512]) so no DVE
bias add is needed. Embedding lookup E[x] on host; layer-0 input
projection + bias is a K=9 matmul chunk [Wih0.T; b0] vs [emb_t; 1].
"""

import sys

sys.path.insert(0, "/opt/trn_rl_repo")

from contextlib import ExitStack

import numpy as np

VOCAB = 78
EMBED = 8
H = 256
BATCH = 512
SEQ = 512
NCORES = 8
BPC = BATCH // NCORES  # 64 batch per core
EBLK = 64  # emb prefetch block (steps)

_cache = {}


def _build_program(T):
    import concourse.tile as tile
    import concourse.mybir as mybir
    from concourse import bacc

    dt = mybir.dt
    AF = mybir.ActivationFunctionType
    ALU = mybir.AluOpType
    f32, f16 = dt.float32, dt.float16

    nc = bacc.Bacc("TRN2", target_bir_lowering=False, debug=False,
                   num_devices=NCORES)

    W0h_d = nc.dram_tensor("W0h", [128, 2, 1024], f16, kind="ExternalInput").ap()
    W0e_d = nc.dram_tensor("W0e", [9, 1024], f16, kind="ExternalInput").ap()
    W1_d = nc.dram_tensor("W1", [128, 4, 1024], f16, kind="ExternalInput").ap()
    B1p_d = nc.dram_tensor("B1p", [8, 128], f16, kind="ExternalInput").ap()
    E8_d = nc.dram_tensor("E8", [8, 512], f16, kind="ExternalInput").ap()
    Wfc_d = nc.dram_tensor("WfcT", [128, 2, VOCAB], f16, kind="ExternalInput").ap()
    bfc_d = nc.dram_tensor("bfc", [VOCAB, 1], f32, kind="ExternalInput").ap()
    emb_d = nc.dram_tensor("embT", [9, T * BPC], f16, kind="ExternalInput").ap()
    out_d = nc.dram_tensor("out", [VOCAB, BPC], f32, kind="ExternalOutput").ap()

    with tile.TileContext(nc) as tc, ExitStack() as ctx:
        const = ctx.enter_context(tc.tile_pool(name="const", bufs=1))
        W0h = const.tile([128, 2, 1024], f16)
        nc.sync.dma_start(W0h[:], W0h_d)
        W0e = const.tile([9, 1024], f16)
        nc.sync.dma_start(W0e[:], W0e_d)
        W1 = const.tile([128, 4, 1024], f16)
        nc.sync.dma_start(W1[:], W1_d)
        B1p = const.tile([8, 128], f16)
        nc.sync.dma_start(B1p[:], B1p_d)
        E8 = const.tile([8, 512], f16)
        nc.sync.dma_start(E8[:], E8_d)
        WfcT = const.tile([128, 2, VOCAB], f16)
        nc.sync.dma_start(WfcT[:], Wfc_d)
        bfc = const.tile([VOCAB, 1], f32)
        nc.sync.dma_start(bfc[:], bfc_d)

        embp = ctx.enter_context(tc.tile_pool(name="embp", bufs=2))
        ps0p = ctx.enter_context(tc.tile_pool(name="ps0p", bufs=3, space="PSUM"))
        ps1p = ctx.enter_context(tc.tile_pool(name="ps1p", bufs=2, space="PSUM"))
        psfcp = ctx.enter_context(tc.tile_pool(name="psfcp", bufs=1, space="PSUM"))
        gp = ctx.enter_context(tc.tile_pool(name="gp", bufs=2))
        op_ = ctx.enter_context(tc.tile_pool(name="op", bufs=2))
        up = ctx.enter_context(tc.tile_pool(name="up", bufs=2))
        tp = ctx.enter_context(tc.tile_pool(name="tp", bufs=2))
        cp = ctx.enter_context(tc.tile_pool(name="cp", bufs=2))
        tcp = ctx.enter_context(tc.tile_pool(name="tcp", bufs=2))
        hp = ctx.enter_context(tc.tile_pool(name="hp", bufs=2))
        fcp = ctx.enter_context(tc.tile_pool(name="fcp", bufs=1))

        eblk = min(EBLK, T)
        nblk = (T + eblk - 1) // eblk
        emb_tiles = [None] * nblk
        ps0_tiles = {}
        ps1_tiles = {}
        h0_prev = h1_prev = None

        def emit_bias(step):
            # ps1 bank for L1 step `step`-1 (wavefront `step`): allocate one
            # wavefront step early and inject b1 via a K=8 matmul so the PE
            # does it warm, inside the previous step's burst.
            ps1 = ps1p.tile([128, 512], f32, name="ps1")
            ps1_tiles[step] = ps1
            nc.tensor.matmul(ps1[:, 0:512], B1p[:, :], E8[:, :],
                             start=True, stop=False, skip_group_check=True)

        def emit_emb(step):
            # layer-0 input-projection matmuls for `step`, into a fresh ps0
            # tile, emitted 2 steps early as PE filler. m0 carries start=True
            # (clears bank has_written); later mms overwrite/accumulate per
            # element.
            ps0 = ps0p.tile([128, 512], f32, name="ps0")
            ps0_tiles[step] = ps0
            emb_sb = emb_tiles[step // eblk]
            erhs = emb_sb[:, (step % eblk) * BPC:(step % eblk + 1) * BPC]
            for m in range(8):
                nc.tensor.matmul(ps0[:, m * 64:(m + 1) * 64],
                                 W0e[:, m * 128:(m + 1) * 128], erhs,
                                 start=(m == 0),
                                 stop=(step == 0 and m == 7),
                                 skip_group_check=True)

        def cell(ps, first, tag):
            # gates (f,i,g,o) after the host row-permutation: tanh(f) first
            # (it alone gates U, the first chain op), then tanh(i,g), then
            # tanh(o) (only needed at the very end of the chain)
            Gf = gp.tile([128, 128], f16, name="Gf" + tag)
            nc.scalar.activation(Gf[:], ps[:, 0:128], AF.Tanh)
            # i,g,o in ONE [384] instr: one less ACT instruction per cell
            # (~250ns fixed cost each) on the most-loaded engine; o sits in
            # the same tile, read at chain end
            Gig = gp.tile([128, 384], f16, name="Gig" + tag)
            nc.scalar.activation(Gig[:], ps[:, 128:512], AF.Tanh)
            O = Gig[:, 256:384]
            C = cp.tile([128, 128], f16, name="C" + tag)
            if first:
                # c = i*g  ->  C = (it+1)*gt
                nc.vector.scalar_tensor_tensor(
                    C[:], Gig[:, 0:128], 1.0, Gig[:, 128:256],
                    ALU.add, ALU.mult)
            else:
                U = up.tile([128, 128], f16, name="U" + tag)
                nc.vector.scalar_tensor_tensor(
                    U[:], Gf[:], 1.0, cell.cprev[tag][:], ALU.add, ALU.mult)
                Tt = tp.tile([128, 128], f16, name="T" + tag)
                nc.vector.scalar_tensor_tensor(
                    Tt[:], Gig[:, 0:128], 1.0, Gig[:, 128:256],
                    ALU.add, ALU.mult)
                nc.vector.scalar_tensor_tensor(
                    C[:], U[:], 0.5, Tt[:], ALU.mult, ALU.add)
            TC = tcp.tile([128, 128], f16, name="TC" + tag)
            nc.scalar.activation(TC[:], C[:], AF.Tanh, scale=0.5)
            Hh = hp.tile([128, 128], f16, name="H" + tag)
            nc.vector.scalar_tensor_tensor(
                Hh[:], O, 1.0, TC[:], ALU.add, ALU.mult)
            cell.cprev[tag] = C
            return Hh

        cell.cprev = {}

        for s in range(T + 1):
            # prefetch emb blocks: block 0 at s=0, block b+1 at start of block b
            if s < T and s % eblk == 0:
                b = s // eblk
                if b == 0:
                    e0 = embp.tile([9, eblk * BPC], f16, name="embblk")
                    nc.sync.dma_start(e0[:], emb_d[:, 0:eblk * BPC])
                    emb_tiles[0] = e0
                if b + 1 < nblk:
                    e1 = embp.tile([9, eblk * BPC], f16, name="embblk")
                    nc.sync.dma_start(
                        e1[:],
                        emb_d[:, (b + 1) * eblk * BPC:(b + 2) * eblk * BPC])
                    emb_tiles[b + 1] = e1

            h0_in = h0_prev  # H0(s-1), consumed by both layers this step
            h1_in = h1_prev  # H1(s-2)

            if s == 0:
                emit_emb(0)
                if T > 1:
                    emit_emb(1)
                emit_bias(1)
            # emb/bias filler emitted at the TOP of the step: they are
            # dependency-free so the in-order PE queue runs them during the
            # H0(s-1) wait, keeping the PE warm across the step boundary.
            if 1 <= s and s + 1 <= T:
                emit_bias(s + 1)
            if s + 2 < T:
                emit_emb(s + 2)

            if s < T:
                # ---- layer 0, step s ----
                ps0 = ps0_tiles.pop(s)
                with tc.high_priority():
                    if s > 0:
                        for m in range(8):
                            o = ps0[:, m * 64:(m + 1) * 64]
                            for k in range(2):
                                nc.tensor.matmul(
                                    o, W0h[:, k, m * 128:(m + 1) * 128],
                                    h0_in[:, k * 64:(k + 1) * 64],
                                    start=False, stop=(m == 7 and k == 1),
                                    skip_group_check=True)
                    h0_prev = cell(ps0, s == 0, "0")

            if s > 0:
                # ---- layer 1, step s-1 (bias already in ps1, injected one
                # wavefront step early) ----
                ps1 = ps1_tiles.pop(s)
                for m in range(8):
                    o = ps1[:, m * 64:(m + 1) * 64]
                    if s > 1:
                        for k in range(2):
                            nc.tensor.matmul(
                                o, W1[:, k, m * 128:(m + 1) * 128],
                                h1_in[:, k * 64:(k + 1) * 64],
                                start=False, stop=False,
                                skip_group_check=True)
                    for k in range(2):
                        nc.tensor.matmul(
                            o, W1[:, 2 + k, m * 128:(m + 1) * 128],
                            h0_in[:, k * 64:(k + 1) * 64],
                            start=False, stop=(m == 7 and k == 1),
                            skip_group_check=True)
                h1_prev = cell(ps1, s == 1, "1")

        # ---- final FC on h1(T-1) ----
        psfc = psfcp.tile([VOCAB, BPC], f32)
        for k in range(2):
            nc.tensor.matmul(psfc[:], WfcT[:, k, :],
                             h1_prev[:, k * 64:(k + 1) * 64],
                             start=(k == 0), stop=(k == 1))
        fc = fcp.tile([VOCAB, BPC], f32)
        nc.scalar.activation(fc[:], psfc[:], AF.Identity, bias=bfc[:])
        nc.sync.dma_start(out_d, fc[:])

    nc.compile()
    return nc


def _prep_inputs(x, E, Wih0, Whh0, bih0, bhh0, Wih1, Whh1, bih1, bhh1,
                 Wfc, bfc, T):
    """Host-side weight folding and per-core input shards."""
    f16 = np.float16
    # permute gate rows (i,f,g,o) -> (f,i,g,o) so tanh(f) is a single leading
    # block (it alone gates the first chain op U)
    perm = np.r_[256:512, 0:256, 512:1024]
    # rowscale: 0.5 for f,i,o gate rows (sigma -> tanh), 1.0 for g rows
    rs = np.ones(1024, np.float32)
    rs[0:512] = 0.5      # f, i
    rs[768:1024] = 0.5   # o
    Wih0 = np.asarray(Wih0, np.float32)[perm] * rs[:, None]
    Whh0 = np.asarray(Whh0, np.float32)[perm] * rs[:, None] * 0.5  # H0 = 2h0
    b0 = (np.asarray(bih0, np.float32) + np.asarray(bhh0, np.float32))[perm] * rs
    Wih1 = np.asarray(Wih1, np.float32)[perm] * rs[:, None] * 0.5  # H0 = 2h0
    Whh1 = np.asarray(Whh1, np.float32)[perm] * rs[:, None] * 0.5  # H1 = 2h1
    b1 = (np.asarray(bih1, np.float32) + np.asarray(bhh1, np.float32))[perm] * rs
    Wfc = np.asarray(Wfc, np.float32) * 0.5                  # input H1 = 2h1
    bfc = np.asarray(bfc, np.float32)

    W0h = np.ascontiguousarray(
        Whh0.T.reshape(2, 128, 1024).transpose(1, 0, 2)).astype(f16)
    W0e = np.concatenate([Wih0.T, b0[None, :]], axis=0).astype(f16)  # [9,1024]
    W1 = np.ascontiguousarray(
        np.concatenate([Whh1.T, Wih1.T], axis=0)  # [512, 1024]
        .reshape(4, 128, 1024).transpose(1, 0, 2)).astype(f16)
    B1p = np.ascontiguousarray(b1.reshape(8, 128)).astype(f16)
    E8 = np.ascontiguousarray(
        np.repeat(np.eye(8, dtype=np.float32), 64, axis=1)).astype(f16)
    WfcT = np.ascontiguousarray(
        Wfc.T.reshape(2, 128, VOCAB).transpose(1, 0, 2)).astype(f16)
    bfc2 = np.ascontiguousarray(bfc[:, None]).astype(np.float32)

    E2 = np.asarray(E, np.float32).copy()
    E2[0] = 0.0  # padding_idx=0
    x = np.asarray(x)

    common = {"W0h": W0h, "W0e": W0e, "W1": W1, "B1p": B1p, "E8": E8,
              "WfcT": WfcT, "bfc": bfc2}
    in_maps = []
    for i in range(NCORES):
        xs = x[i * BPC:(i + 1) * BPC, :T]  # [64, T]
        emb = E2[xs]  # [64, T, 8]
        embT = np.empty((9, T, BPC), np.float32)
        embT[:8] = emb.transpose(2, 1, 0)
        embT[8] = 1.0
        m = dict(common)
        m["embT"] = np.ascontiguousarray(embT.reshape(9, T * BPC)).astype(f16)
        in_maps.append(m)
    return in_maps


def kernel(x, E, Wih0, Whh0, bih0, bhh0, Wih1, Whh1, bih1, bhh1, Wfc, bfc,
           T=SEQ, trace=False):
    from concourse import bass_utils

    if T not in _cache:
        _cache[T] = _build_program(T)
    nc = _cache[T]
    in_maps = _prep_inputs(x, E, Wih0, Whh0, bih0, bhh0, Wih1, Whh1, bih1,
                           bhh1, Wfc, bfc, T)
    res = bass_utils.run_bass_kernel_spmd(nc, in_maps, list(range(NCORES)),
                                          trace=trace)
    out = np.empty((BATCH, VOCAB), np.float32)
    for i in range(NCORES):
        out[i * BPC:(i + 1) * BPC] = np.asarray(res.results[i]["out"]).T
    if trace:
        return out, res
    return out
